# revision 75
# baseline (speedup 1.0000x reference)
"""Trainium2 Bass kernel for nn_Attention_spd (dense transformer attention with
pairwise score bias `spd`, head-drop rescale, and output projection).

Reference computation (b=4, n=1024, dim=512, heads=8, dim_head=64):
    qkv = x @ w_qkv ; q,k,v = split
    dots = q @ k^T * scale + spd
    attn = softmax(dots) * (head_keep * H / sum(head_keep))
    out  = (attn @ v) @ w_out + b_out

Sharding across 8 NeuronCores: core c handles batch c//2 and heads
4*(c%2) .. 4*(c%2)+3 (data parallel on batch x tensor parallel on heads).
Each core computes a partial output projection over its 4 heads; the host
sums the two partials per batch (cheap 2-way reduce) and adds b_out.

Device-side design (v2 — flipped attention/projection):
  - x / w_qkv / w_out shipped bf16 (halves the weight+activation DMA);
    q/k kept f32r on-chip for the dots; attention probabilities bf16.
  - dots computed transposed: pd[j, (s,i)] = k_s @ q_s^T per 128-j block.
  - exp(dots + spd) = exp(dots) * exp(spd); exp(spd) precomputed on host in
    bf16; combine is a bf16 DVE multiply (two per phase offloaded to gpsimd).
  - attn@v FLIPPED: pr chunk [128j, 128i] is the *stationary* operand and
    v_aug [128j, 65] (v columns + ones) the moving operand, so each matmul
    streams only 65 columns (matmul cost is moving-size only).  Output
    po2[i, 65] carries the softmax denominator in column 64, which makes the
    normalization a per-partition reciprocal + broadcast multiply (no
    partition-broadcast matmul chain).
  - normalized attention (bf16) is transposed back to [head*d, i] layout with
    PE identity-transposes into PSUM + cheap bf16 copies out.
  - output projection FLIPPED: w_out chunks [128hd, 128m] stationary,
    scaled attention [128hd, i] moving -> y^T [m, i] (bf16 partials),
    halving projection matmul cycles; host un-transposes and sums.
  - phase pipeline: phase P's dots/exp/mult overlap phase P-1's attn@v
    (bursts in jb slots 1-4), norms (per ic-pair), transposes (jb5/6) and
    the previous ib's projection (jb0/1).  The final phase flushes through
    wide-stride PSUM tiles so all accumulation groups interleave, with the
    last exp/mult split by head to shorten the tail.
  - scale folded into wq on host; head_keep folded into w_out rows.
  - PE warm-up burst at t~0 against a gpsimd-memset tile (the cost model's
    p-state ramp needs ~3us of activity before full clock).
"""
import os
import sys

for _p in ("/opt/trn_rl_repo", os.path.expanduser("~/.axon_site/_ro/trn_rl_repo")):
    if os.path.isdir(_p) and _p not in sys.path:
        sys.path.insert(0, _p)

import numpy as np
import ml_dtypes

import concourse.bass as bass  # noqa: F401
import concourse.tile as tile
from concourse import bacc, mybir
from concourse.bass_utils import run_bass_kernel_spmd

P = 128
B, N, DIM = 4, 1024, 512
HEADS = 8
DIM_HEAD = 64
SCALE = DIM_HEAD ** -0.5
HL = 4          # heads per core (local)
F32 = mybir.dt.float32
F32R = mybir.dt.float32r
BF16 = mybir.dt.bfloat16
ADD = mybir.AluOpType.add
MULT = mybir.AluOpType.mult
EXP = mybir.ActivationFunctionType.Exp

VARIANT = "flip"

_NC = {}

# jb indices whose exp(spd) multiply runs on gpsimd (Pool) instead of DVE
POOL_JBS = (0, 3)


def build_nc(variant=VARIANT):
    """Build the SPMD Bass program (identical on all 8 cores)."""
    nc = bacc.Bacc("TRN2", target_bir_lowering=False, debug=False, num_devices=8)
    xT = nc.dram_tensor("xT", [DIM, N], BF16, kind="ExternalInput").ap()
    # [qm0 | km0 | v | qm1 | km1] so a small early DMA unblocks the first phase
    w3 = nc.dram_tensor("w3", [DIM, 3 * HL * DIM_HEAD], BF16, kind="ExternalInput").ap()
    # [ (s,d), hp, mc, m ] : projection weights, row-scaled by head_keep
    wo2 = nc.dram_tensor("wo2", [P, 1024], BF16, kind="ExternalInput").ap()
    # exp(spd) in bf16: [hp, ib, jj, jb, s, ii] — per (hp, ib) contiguous
    spdT = nc.dram_tensor("spdT", [2, 2, P, 8, 2, 512], BF16, kind="ExternalInput").ap()
    # identity for PE-transpose of the flipped attention output
    id128 = nc.dram_tensor("id128", [P, P], BF16, kind="ExternalInput").ap()
    # y^T partial: [mc, m, i] (bf16; host upcasts before the partial-sum)
    y = nc.dram_tensor("y", [4, P, N], BF16, kind="ExternalOutput").ap()

    from contextlib import ExitStack

    with tile.TileContext(nc) as tc, ExitStack() as ctx:
        sb = ctx.enter_context(tc.tile_pool(name="sb", bufs=1))
        spd_pool = ctx.enter_context(tc.tile_pool(name="spd", bufs=4))
        ex_pool = ctx.enter_context(tc.tile_pool(name="ex", bufs=4))
        pr_pool = ctx.enter_context(tc.tile_pool(name="pr", bufs=16))
        scp_pool = ctx.enter_context(tc.tile_pool(name="scp", bufs=2))
        r_pool = ctx.enter_context(tc.tile_pool(name="r", bufs=4))
        sc2_pool = ctx.enter_context(tc.tile_pool(name="sc2", bufs=2))
        ps = ctx.enter_context(tc.tile_pool(name="ps", bufs=2, space="PSUM"))
        po_ps = ctx.enter_context(tc.tile_pool(name="po", bufs=2, space="PSUM"))
        ps4 = ctx.enter_context(tc.tile_pool(name="ps4", bufs=2, space="PSUM"))

        # ---- warm-up (PE p-state ramp) before any DMA-dependent work -------
        warm_sb = sb.tile([P, 384], BF16, tag="warm")
        nc.gpsimd.memset(warm_sb[:], 1.0)
        # identity on the otherwise-idle ACT HWDGE queue
        id_sb = sb.tile([P, P], BF16, tag="id")
        nc.scalar.dma_start(id_sb[:], id128[:])
        for w in range(14):
            pw = ps4.tile([P, 256], F32, tag="s4", name=f"warm_{w}")
            nc.tensor.matmul(pw[:], warm_sb[:, 0:128], warm_sb[:, 128:384],
                             start=True, stop=True)

        # ---- resident loads ------------------------------------------------
        xT_sb = sb.tile([P, 4, N], BF16)
        w3_sb = sb.tile([P, 4, 768], BF16, tag="w3")
        wo2_sb = sb.tile([P, 2, 4, P], BF16, tag="wo2")
        xT_r = xT.rearrange("(kb p) n -> p kb n", p=P)
        w3_r = w3.rearrange("(kb p) m -> p kb m", p=P)
        # per-kb x transfers so the qkv matmuls can chase the DMA
        nc.sync.dma_start(xT_sb[:, 0, :], xT_r[:, 0, :])
        nc.sync.dma_start(w3_sb[:, :, 0:256], w3_r[:, :, 0:256])      # q/k m0
        nc.sync.dma_start(xT_sb[:, 1, :], xT_r[:, 1, :])
        nc.sync.dma_start(xT_sb[:, 2, :], xT_r[:, 2, :])
        nc.sync.dma_start(xT_sb[:, 3, :], xT_r[:, 3, :])

        # v with ones column: [j, jb, hl, 65]
        v_aug = sb.tile([P, 8, HL, 65], BF16, tag="vaug")
        nc.gpsimd.memset(v_aug[:, :, :, 64:65], 1.0)

        qT_sb = sb.tile([P, 2, N], F32R, tag="qT")
        kT_sb = sb.tile([P, 2, N], F32R, tag="kT")

        # ---- q/k m0 projections (chasing the per-kb x DMA) -----------------
        # token-half granularity: the first dots only need tokens 0-511 of
        # q (moving) and k (stationary), so copy each half out immediately
        def qk_half(qk, nb, dst):
            wofs = qk * 128
            pq = ps4.tile([P, 512], F32, tag="s4", name=f"pq_{qk}_{nb}")
            for kb in range(4):
                nc.tensor.matmul(
                    pq[:],
                    w3_sb[:, kb, wofs:wofs + 128],
                    xT_sb[:, kb, nb * 512:(nb + 1) * 512],
                    start=(kb == 0),
                    stop=(kb == 3),
                )
            if (qk, nb) == (1, 0):
                # first dots only need k tokens 0-127
                nc.vector.tensor_copy(dst[:, 0, 0:128], pq[:, 0:128])
                nc.vector.tensor_copy(dst[:, 0, 128:512], pq[:, 128:512])
            else:
                nc.vector.tensor_copy(dst[:, 0, nb * 512:(nb + 1) * 512], pq[:])

        qk_half(0, 0, qT_sb)
        qk_half(1, 0, kT_sb)

        # first attention phase's spd goes ahead of the remaining weights
        st00 = spd_pool.tile([P, 8, 2, 512], BF16, tag="spd", name="spd_0_0")
        nc.sync.dma_start(st00[:, 0:2], spdT[0, 0, :, 0:2])
        nc.sync.dma_start(w3_sb[:, :, 256:512], w3_r[:, :, 256:512])  # v
        nc.sync.dma_start(st00[:, 2:4], spdT[0, 0, :, 2:4])
        nc.sync.dma_start(st00[:, 4:6], spdT[0, 0, :, 4:6])
        nc.sync.dma_start(st00[:, 6:8], spdT[0, 0, :, 6:8])
        nc.sync.dma_start(w3_sb[:, :, 512:768], w3_r[:, :, 512:768])  # q/k m1
        nc.sync.dma_start(wo2_sb[:], wo2.rearrange("p (hp mc m) -> p hp mc m",
                                                   hp=2, mc=4))

        # ---- helpers -------------------------------------------------------
        y_sb = sb.tile([P, 4, 2, 512], BF16, tag="ysb")

        pv_cur = [None]

        def v_chunk_half(jb):
            # v projection for one token block; copy out per pair
            half = jb % 2
            if half == 0:
                pv_cur[0] = ps4.tile([P, 512], F32, tag="s4", name=f"pv_{jb//2}")
            pv = pv_cur[0]
            for kb in range(4):
                nc.tensor.matmul(
                    pv[:, half * 256:half * 256 + 256],
                    xT_sb[:, kb, jb * 128:(jb + 1) * 128],
                    w3_sb[:, kb, 256:512],
                    start=(kb == 0),
                    stop=(kb == 3),
                )
            if half == 1:
                nc.vector.tensor_copy(
                    v_aug[:, jb - 1:jb + 1, :, 0:64],
                    pv[:].rearrange("p (half h c) -> p half h c", half=2, c=64),
                )

        def transpose_pair(nc, scp, sc2, hp, pair, pool=None, tag="s4",
                           mix_eng=False, shared=False):
            # PE-transpose [i, (s,d)] -> [(s,d), i] via identity matmul, then
            # a cheap bf16 copy PSUM->SBUF (ACT/DVE alternated in the flush)
            pts = None
            if shared:
                # one pool slot serves both ics: the second pair needs no
                # ring rotation (and so no wait on the first pair's copies)
                pts = (pool or ps4).tile([P, 512], F32, tag=tag,
                                         name=f"pts_{hp}_{pair}")
            for k, ic in enumerate((2 * pair, 2 * pair + 1)):
                if shared:
                    pt = pts[:, 64 * k:64 * k + 64]
                else:
                    pt = (pool or ps4).tile([P, 512], F32, tag=tag,
                                            name=f"pt_{hp}_{pair}_{ic}")[:, 0:64]
                ptv = pt.bitcast(BF16)
                nc.tensor.transpose(
                    ptv,
                    scp[:, ic, :, :].rearrange("p s d -> p (s d)"),
                    id_sb[:],
                )
                if mix_eng and k == 0:
                    nc.scalar.copy(sc2[:, hp, ic * 128:(ic + 1) * 128], ptv)
                else:
                    nc.vector.tensor_copy(sc2[:, hp, ic * 128:(ic + 1) * 128],
                                          ptv)

        def m1_chunk(qk, nb, dst):
            wofs = 512 + qk * 128
            pq1 = ps4.tile([P, 512], F32, tag="s4", name=f"pq1_{qk}_{nb}")
            for kb in range(4):
                nc.tensor.matmul(
                    pq1[:],
                    w3_sb[:, kb, wofs:wofs + 128],
                    xT_sb[:, kb, nb * 512:(nb + 1) * 512],
                    start=(kb == 0),
                    stop=(kb == 3),
                )
            nc.vector.tensor_copy(dst[:, 1, nb * 512:(nb + 1) * 512], pq1[:])

        def proj(ib, sc2, flush=False, mcs=range(4)):
            # flipped projection: y^T[m, i] accumulated over both head pairs.
            # In the flush the ACT engine is idle (exps done) — use it for the
            # PSUM->SBUF copies so they overlap the DVE norm/transpose work.
            for mc in mcs:
                py = ps4.tile([P, 512], F32, tag="s4", name=f"py_{ib}_{mc}")
                for hp in range(2):
                    nc.tensor.matmul(
                        py[:],
                        wo2_sb[:, hp, mc, :],
                        sc2[:, hp, :],
                        start=(hp == 0),
                        stop=(hp == 1),
                    )
                if flush and mc % 2 == 0:
                    nc.scalar.copy(y_sb[:, mc, ib, :], py[:])
                else:
                    nc.vector.tensor_copy(y_sb[:, mc, ib, :], py[:])
                if mc % 2 == 1:
                    # ship each mc-pair as soon as its copies land
                    nc.gpsimd.dma_start(
                        y[mc - 1:mc + 1, :, ib * 512:(ib + 1) * 512]
                        .rearrange("mc p n -> p mc n"),
                        y_sb[:, mc - 1:mc + 1, ib, :])

        # ---- attention phases ---------------------------------------------
        # state for the previous phase whose attn@v runs during this phase
        prev = None          # (pr_list, po2 pair, hp, ib, scp, r pair, sc2)
        pend_proj = None     # ib whose projection is ready to issue

        for ib in range(2):
            sc2 = sc2_pool.tile([P, 2, 512], BF16, tag="sc2", name=f"sc2_{ib}")
            for hp in range(2):
                pidx = 2 * ib + hp
                if pidx == 0:
                    st = st00
                else:
                    st = spd_pool.tile([P, 8, 2, 512], BF16, tag="spd",
                                       name=f"spd_{hp}_{ib}")
                    for q in range(4):
                        nc.sync.dma_start(st[:, 2 * q:2 * q + 2],
                                          spdT[hp, ib, :, 2 * q:2 * q + 2])

                pr_list = []
                po2 = None
                if pidx < 3:
                    po2 = [po_ps.tile([P, 4, 65], F32, tag="po",
                                      name=f"po_{hp}_{ib}_{h}")
                           for h in range(2)]
                scp = scp_pool.tile([P, 4, 2, 64], BF16, tag="scp",
                                    name=f"scp_{hp}_{ib}")
                rr = [r_pool.tile([P, 4], F32, tag="r", name=f"r_{hp}_{ib}_{h}")
                      for h in range(2)]

                for jb in range(8):
                    # -- dots for this phase --
                    pd = ps.tile([P, 1024], F32, tag="big", name=f"pd_{hp}_{ib}_{jb}")
                    for s in range(2):
                        nc.tensor.matmul(
                            pd[:, s * 512:(s + 1) * 512],
                            kT_sb[64 * s:64 * s + 64, hp, jb * 128:(jb + 1) * 128],
                            qT_sb[64 * s:64 * s + 64, hp, ib * 512:(ib + 1) * 512],
                            start=True,
                            stop=True,
                        )
                    # fill work issues before the mult so its DVE copies
                    # aren't queued behind this slot's pr-multiply
                    if prev is None and pidx == 0:
                        if jb == 0:
                            qk_half(1, 1, kT_sb)
                        elif jb == 1:
                            qk_half(0, 1, qT_sb)
                        elif jb < 6:
                            v_chunk_half(2 * (jb - 2))
                            v_chunk_half(2 * (jb - 2) + 1)
                        elif jb == 6:
                            m1_chunk(1, 0, kT_sb)
                        else:
                            m1_chunk(0, 0, qT_sb)
                    if pidx == 1 and jb == 0:
                        m1_chunk(1, 1, kT_sb)
                    if pidx == 1 and jb == 7:
                        m1_chunk(0, 1, qT_sb)
                    ex = ex_pool.tile([P, 1024], BF16, tag="ex",
                                      name=f"ex_{hp}_{ib}_{jb}")
                    pr = pr_pool.tile([P, 2, 512], BF16, tag="pr",
                                      name=f"pr_{hp}_{ib}_{jb}")
                    if pidx == 3 and jb == 7:
                        # final tile: split by head so the flush attn@v can
                        # chase the first half out of the exp
                        for s in range(2):
                            nc.scalar.activation(ex[:, s * 512:(s + 1) * 512],
                                                 pd[:, s * 512:(s + 1) * 512],
                                                 EXP)
                            nc.vector.tensor_tensor(
                                pr[:, s, :], ex[:, s * 512:(s + 1) * 512],
                                st[:, jb, s], MULT)
                    else:
                        nc.scalar.activation(ex[:], pd[:], EXP)
                        eng = nc.gpsimd if jb in POOL_JBS else nc.vector
                        eng.tensor_tensor(
                            pr[:].rearrange("p s i -> p (s i)"), ex[:],
                            st[:, jb].rearrange("p s i -> p (s i)"),
                            MULT,
                        )
                    pr_list.append(pr)
                    if jb in (0, 1) and pend_proj is not None:
                        proj(*pend_proj, mcs=(range(2) if jb == 0 else range(2, 4)))
                        if jb == 1:
                            pend_proj = None

                    # -- interleaved work from the previous phase, packed into
                    # jb slots 1-7 so the next phase's dots start unimpeded --
                    if prev is not None:
                        p_pr, p_po2, p_hp, p_ib, p_scp, p_rr, p_sc2 = prev
                        if 1 <= jb <= 4:
                            h, icp = (jb - 1) // 2, (jb - 1) % 2
                            attnv(nc, p_pr, p_po2, p_hp, h, 2 * icp, v_aug)
                            attnv(nc, p_pr, p_po2, p_hp, h, 2 * icp + 1, v_aug)
                            norm_pair(nc, p_po2, p_scp, p_rr, h, icp)
                        elif jb == 5:
                            transpose_pair(nc, p_scp, p_sc2, p_hp, 0)
                        elif jb == 6:
                            transpose_pair(nc, p_scp, p_sc2, p_hp, 1)
                            if p_hp == 1:
                                pend_proj = (p_ib, p_sc2)



                prev = (pr_list, po2, hp, ib, scp, rr, sc2)

        # ---- flush: last phase's attn@v, norm, transpose, projection -------
        p_pr, p_po2, p_hp, p_ib, p_scp, p_rr, p_sc2 = prev
        # early projection accumulation over the already-transposed head pair
        # (hp0 of ib1) while the PE waits for the last pr tiles
        pys = []
        for mc in range(4):
            pool, tag = (po_ps, "po") if mc < 2 else (ps4, "s4")
            py = pool.tile([P, 512], F32, tag=tag, name=f"pyf_{mc}")
            nc.tensor.matmul(py[:], wo2_sb[:, 0, mc, :], p_sc2[:, 0, :],
                             start=True, stop=False)
            pys.append(py)
        # last phase's attn@v in dots-ring tiles with 1KB ic-stride: ic pairs
        # (0,1) / (2,3) share a PSUM zero region, so waves {0,2} then {1,3}
        # interleave across h and finish right after the last pr lands
        po2f = [ps.tile([P, 4, 256], F32, tag="big", name=f"pof_{h}")
                for h in range(2)]
        for wave in range(2):
            for jbb in range(8):
                for h in range(2):
                    for ic in (wave, wave + 2):
                        nc.tensor.matmul(
                            po2f[h][:, ic, 0:65],
                            p_pr[jbb][:, h, ic * 128:(ic + 1) * 128],
                            v_aug[:, jbb, 2 * p_hp + h, :],
                            start=(jbb == 0),
                            stop=(jbb == 7),
                        )
        with nc.allow_low_precision(reason="f32 recip for softmax denom"):
            for h in range(2):
                nc.vector.reciprocal(p_rr[h][:], po2f[h][:, :, 64])
        for h in range(2):
            nc.vector.tensor_tensor(
                p_scp[:, :, h, :], po2f[h][:, :, 0:64],
                p_rr[h][:, :, None].to_broadcast((P, 4, 64)), MULT)
        transpose_pair(nc, p_scp, p_sc2, p_hp, 0, pool=ps, tag="big",
                       mix_eng=True, shared=True)
        transpose_pair(nc, p_scp, p_sc2, p_hp, 1, pool=ps, tag="big",
                       mix_eng=True, shared=True)
        for mc in range(4):
            nc.tensor.matmul(pys[mc][:], wo2_sb[:, 1, mc, :], p_sc2[:, 1, :],
                             start=False, stop=True)
            if mc % 2 == 0:
                nc.scalar.copy(y_sb[:, mc, p_ib, :], pys[mc][:])
            else:
                nc.vector.tensor_copy(y_sb[:, mc, p_ib, :], pys[mc][:])
                dma_eng = nc.sync if mc == 1 else nc.scalar
                dma_eng.dma_start(
                    y[mc - 1:mc + 1, :, p_ib * 512:(p_ib + 1) * 512]
                    .rearrange("mc p n -> p mc n"),
                    y_sb[:, mc - 1:mc + 1, p_ib, :])

    nc.compile()
    return nc


def attnv(nc, pr_list, po2, hp, h, ic, v_aug):
    """Flipped attn@v: pr chunk stationary, v_aug moving (65 cols)."""
    for jbb in range(8):
        nc.tensor.matmul(
            po2[h][:, ic, :],
            pr_list[jbb][:, h, ic * 128:(ic + 1) * 128],
            v_aug[:, jbb, 2 * hp + h, :],
            start=(jbb == 0),
            stop=(jbb == 7),
        )


def norm_h(nc, po2, scp, rr, h):
    """Normalize all four i-chunks of one head (col 64 = denominator)."""
    with nc.allow_low_precision(reason="f32 recip is plenty for softmax denom"):
        nc.vector.reciprocal(rr[h][:], po2[h][:, :, 64])
    nc.vector.tensor_tensor(
        scp[:, :, h, :],
        po2[h][:, :, 0:64],
        rr[h][:, :, None].to_broadcast((P, 4, 64)),
        MULT,
    )


def norm_pair(nc, po2, scp, rr, h, pair, act_mult=False):
    """Normalize an ic-pair of the flipped attention output (col 64 = denom)."""
    sl = slice(2 * pair, 2 * pair + 2)
    with nc.allow_low_precision(reason="f32 recip is plenty for softmax denom"):
        nc.vector.reciprocal(rr[h][:, sl], po2[h][:, sl, 64])
    if act_mult:
        # per-ic scalar multiplies on the ACT engine (idle during the flush)
        for ic in (2 * pair, 2 * pair + 1):
            nc.scalar.mul(scp[:, ic, h, :], po2[h][:, ic, 0:64],
                          rr[h][:, ic:ic + 1])
    else:
        nc.vector.tensor_tensor(
            scp[:, sl, h, :],
            po2[h][:, sl, 0:64],
            rr[h][:, sl, None].to_broadcast((P, 2, 64)),
            MULT,
        )


def _get_nc(variant=VARIANT):
    if variant not in _NC:
        _NC[variant] = build_nc(variant)
    return _NC[variant]


def make_in_maps(x, spd, head_keep, w_qkv, w_out, variant=VARIANT):
    x = np.asarray(x, np.float32)
    spd = np.asarray(spd, np.float32)
    keep = np.asarray(head_keep, np.float32)
    w_qkv = np.asarray(w_qkv, np.float32)
    w_out = np.asarray(w_out, np.float32)
    cfac = keep * (HEADS / keep.sum())

    in_maps = []
    for c in range(8):
        bi, hh = divmod(c, 2)
        h0 = hh * HL
        hs = slice(h0 * DIM_HEAD, (h0 + HL) * DIM_HEAD)
        xT = np.ascontiguousarray(x[bi].T.astype(ml_dtypes.bfloat16))
        q_cols = w_qkv[:, hs] * np.float32(SCALE)
        k_cols = w_qkv[:, DIM + h0 * DIM_HEAD:DIM + (h0 + HL) * DIM_HEAD]
        v_cols_h = w_qkv[:, 2 * DIM + h0 * DIM_HEAD:2 * DIM + (h0 + HL) * DIM_HEAD]
        w3 = np.ascontiguousarray(np.concatenate(
            [q_cols[:, :128], k_cols[:, :128], v_cols_h,
             q_cols[:, 128:], k_cols[:, 128:]],
            axis=1,
        ).astype(ml_dtypes.bfloat16))
        wo_rows = w_out[hs, :] * np.repeat(cfac[h0:h0 + HL], DIM_HEAD)[:, None]
        # [ (hp, s, d), m ] -> [ (s, d), hp, mc, m ]
        wo2 = wo_rows.reshape(2, 2, DIM_HEAD, 4, 128).transpose(1, 2, 0, 3, 4)
        wo2 = np.ascontiguousarray(wo2.reshape(P, 1024).astype(ml_dtypes.bfloat16))
        sp = spd[bi, h0:h0 + HL]  # [HL, i, j] with h = 2*hp + s
        # [hp, s, ib, ii, jb, jj] -> [hp, ib, jj, jb, s, ii]
        spdT = sp.reshape(2, 2, 2, 512, 8, 128).transpose(0, 2, 5, 4, 1, 3)
        spdT = np.exp(spdT).astype(ml_dtypes.bfloat16)
        in_maps.append({"xT": xT, "w3": w3, "wo2": wo2,
                        "spdT": np.ascontiguousarray(spdT),
                        "id128": np.eye(P, dtype=ml_dtypes.bfloat16)})
    return in_maps


def kernel(x, spd, head_keep, w_qkv, w_out, b_out):
    assert x.shape == (B, N, DIM) and spd.shape == (B, HEADS, N, N)
    nc = _get_nc()
    in_maps = make_in_maps(x, spd, head_keep, w_qkv, w_out)
    res = run_bass_kernel_spmd(nc, in_maps, core_ids=list(range(8)))
    out = np.empty((B, N, DIM), np.float32)
    for bi in range(B):
        yT = (res.results[2 * bi]["y"].astype(np.float32)
              + res.results[2 * bi + 1]["y"].astype(np.float32))
        out[bi] = yT.reshape(DIM, N).T
    out += np.asarray(b_out, np.float32)[None, None, :]
    return out
